# revision 1
# baseline (speedup 1.0000x reference)
"""Trainium2 Bass kernel for nn_DecoderRNN (soft-attention LSTM decoder).

Sharding (8 NeuronCores, one chip):
  - Attention (relu-add / scores / softmax / context): data-parallel over
    batch, 16 batches per core.  att-precomputed features stay SBUF-resident.
  - LSTM gates: tensor-parallel over hidden dim (128 h per core).
  - Vocab projection W_out: tensor-parallel over vocab (1250 v per core).
  - Per step: AllGather of context [B,VD] and of hx^T [H,B] across cores.

Layouts (per core):
  - A''  = |w| * (features @ att_vw_W.T + att_bias)  : [VD(4x128p), b*224+n] bf16
  - full'' = relu(A'' + |w|*att_h broadcast)          : same layout, bf16
  - scores = sum_e sign(w)[e] * full''[e, b, n]       : PE matvec (M=16 replicated)
  - context: per-b matvec over n, PE column-group packed 4-wide
  - gates/W_out: activations feature-major (xT) stationary, weights stream;
    bf16 matmul inputs, fp32 psum/pointwise state.
"""
import sys, os
sys.path.insert(0, "/opt/trn_rl_repo")

import numpy as np
import ml_dtypes

B, N, VD, H, E, V, T_FULL = 128, 196, 512, 1024, 512, 10000, 40
NCORES = 8
BL = B // NCORES          # 16 batches per core
NP = 224                  # padded n for free-dim layouts (16 chunks of 448 = 7x512-ish)
BNF = BL * NP             # 3584 free size of A''/full''
VL = 1250                 # true vocab per core
VLP = 1280                # padded vocab per core
HL = H // NCORES          # 128 hidden per core

BF16 = ml_dtypes.bfloat16


# ----------------------------------------------------------------------------
# device program
# ----------------------------------------------------------------------------

def build_program(tc, io, T):
    import concourse.bass as bass
    from concourse import mybir

    nc = tc.nc
    F32 = mybir.dt.float32
    BF = mybir.dt.bfloat16
    AF = mybir.ActivationFunctionType
    OP = mybir.AluOpType

    def f32r(ap):
        return ap.bitcast(mybir.dt.float32r)

    # ---------------- static SBUF tensors ----------------
    _statics_cm = tc.tile_pool(name="statics", bufs=1)
    _statics = _statics_cm.__enter__()

    def sb(name, shape, dt):
        return _statics.tile(shape, dt, name=name, tag=name)

    A_sb = sb("A_sb", [128, 4 * BNF], BF)
    full_sb = sb("full_sb", [128, 4 * BNF], BF)
    fea_sb = sb("fea_sb", [128, 16384], BF)
    wcat_sb = sb("wcat_sb", [128, 16 * 512], BF)
    wout_sb = sb("wout_sb", [128, 8 * VLP], BF)
    hwfold_sb = sb("hwfold_sb", [128, 8 * 512], BF)
    wsgn_sb = sb("wsgn_sb", [128, 1024], BF)
    hxT_sb = sb("hxT_sb", [128, 1024], F32)
    hxT_bf = sb("hxT_bf", [128, 1024], BF)
    fmT_sb = sb("fmT_sb", [128, 512], BF)
    ctxT_sb = sb("ctxT_sb", [128, 512], BF)
    ctx_full_sb = sb("ctx_full_sb", [128, 512], F32)
    zcol_sb = sb("zcol_sb", [1, 128], BF)
    out_sb = sb("out_sb", [128, VLP], F32)
    attT_sb = sb("attT_sb", [128, 64], F32)
    hxTp_sb = sb("hxTp_sb", [128, 128], F32)
    alpha_sb = sb("alpha_sb", [16, NP], BF)
    sumexp_sb = sb("sumexp_sb", [16, 1], F32)
    rs_sb = sb("rs_sb", [16, 1], F32)
    aT1_sb = sb("aT1_sb", [128, 16], BF)
    aT2_sb = sb("aT2_sb", [128, 16], BF)
    sig_sb = sb("sig_sb", [128, 384], F32)
    gt_sb = sb("gt_sb", [128, 128], F32)
    tmp_sb = sb("tmp_sb", [128, 128], F32)
    tcx_sb = sb("tcx_sb", [128, 128], F32)
    hx_sb = sb("hx_sb", [128, 128], F32)
    cx_sb = sb("cx_sb", [128, 128], F32)
    eye32_sb = sb("eye32_sb", [128, 128], F32)
    eye16_sb = sb("eye16_sb", [16, 16], BF)
    ones_sb = sb("ones_sb", [1, 128], BF)
    uni_sb = sb("uni_sb", [128, 1], BF)
    bout_sb = sb("bout_sb", [1, VLP], BF)
    gbias_sb = sb("gbias_sb", [1, 512], BF)

    # ---------------- internal DRAM for collectives ----------------
    ag_ctx_in = nc.dram_tensor("ag_ctx_in", [BL, 512], F32, kind="Internal").ap()
    ag_ctx_out = nc.dram_tensor("ag_ctx_out", [128, 512], F32, kind="Internal",
                                addr_space="Shared").ap()
    ag_hx_in = nc.dram_tensor("ag_hx_in", [128, 128], F32, kind="Internal").ap()
    ag_hx_out = nc.dram_tensor("ag_hx_out", [1024, 128], F32, kind="Internal",
                               addr_space="Shared").ap()

    RG = [list(range(NCORES))]

    # ---------------- pools ----------------
    _pool_cms = []

    def open_pool(**kw):
        cm = tc.tile_pool(**kw)
        pool = cm.__enter__()
        _pool_cms.append(cm)
        return pool

    psum_mm = open_pool(name="psum_mm", bufs=2, space="PSUM")
    psum_ctx = open_pool(name="psum_ctx", bufs=2, space="PSUM")
    psum_sc = open_pool(name="psum_sc", bufs=2, space="PSUM")
    psum_misc = open_pool(name="psum_misc", bufs=2, space="PSUM")
    emb_pool = open_pool(name="emb_pool", bufs=2)
    scr_pool = open_pool(name="scr_pool", bufs=2)

    # ---------------- init: weight DMAs + memsets ----------------
    nc.sync.dma_start(fea_sb[:, :], io["fea"][:, :])
    nc.sync.dma_start(wcat_sb[:, :], io["wcat"][:, :])
    nc.sync.dma_start(wout_sb[:, :], io["wout"][:, :])
    nc.sync.dma_start(hwfold_sb[:, :], io["hwfold"][:, :])
    nc.sync.dma_start(wsgn_sb[:, :], io["wsgn16"][:, :])
    nc.sync.dma_start(eye32_sb[:, :], io["eye32"][:, :])
    nc.sync.dma_start(eye16_sb[:, :], io["eye16"][:, :])
    nc.sync.dma_start(bout_sb[:, :], io["bout"][:, :])
    nc.sync.dma_start(gbias_sb[:, :], io["gbias"][:, :])

    nc.vector.memset(hxT_sb[:, :], 0.0)
    nc.vector.memset(hxT_bf[:, :], 0.0)
    nc.vector.memset(cx_sb[:, :], 0.0)
    nc.vector.memset(full_sb[:, :], 0.0)
    nc.vector.memset(alpha_sb[:, :], 0.0)
    nc.vector.memset(ones_sb[:, :], 1.0)
    nc.vector.memset(uni_sb[:, :], 1.0 / N)
    nc.vector.memset(zcol_sb[:, :], 0.0)

    pid_pe = nc.tensor.partition_id()

    def ctx_rounds(lhs1, lhs2):
        """16 per-b matvecs over n (contract 196), 4 col-groups x 4 rounds.
        lhs1/lhs2: function b -> AP ([128,1] and [68,1] stationary vectors).
        Result rows land in psum rows 32j; copied to ctx_scr then DMA'd to
        ag_ctx_in rows 4r..4r+4."""
        for r in range(4):
            pc = psum_ctx.tile([128, 512], F32, name="pc", tag="pc")
            # K=1 zero matmul initializes every psum row (rows other than
            # 32j stay zero; sim + verifier both need them written)
            nc.tensor.matmul(pc[:, :], zcol_sb[0:1, :], fea_sb[0:1, 0:512],
                             start=True, stop=False, skip_group_check=True)
            for j in range(4):
                b = 4 * r + j
                nc.tensor.matmul(pc[32 * j:32 * j + 1, :], lhs1(b),
                                 fea_sb[0:128, (2 * b) * 512:(2 * b + 1) * 512],
                                 start=False, stop=False, tile_position=(0, 32 * j),
                                 skip_group_check=True)
                nc.tensor.matmul(pc[32 * j:32 * j + 1, :], lhs2(b),
                                 fea_sb[0:68, (2 * b + 1) * 512:(2 * b + 1) * 512 + 512],
                                 start=False, stop=(j == 3), tile_position=(0, 32 * j),
                                 skip_group_check=True)
            scr = scr_pool.tile([128, 512], F32, name="scr", tag="scr")
            if r % 2 == 0:
                nc.vector.tensor_copy(scr[:, :], pc[:, :])
            else:
                nc.scalar.copy(scr[:, :], pc[:, :])
            nc.sync.dma_start(ag_ctx_in[4 * r:4 * r + 4, :], scr[0:128:32, :])
        nc.gpsimd.collective_compute(
            "AllGather", OP.bypass, replica_groups=RG,
            ins=[ag_ctx_in[:, :]], outs=[ag_ctx_out[:, :]])
        nc.sync.dma_start(ctx_full_sb[:, :], ag_ctx_out[:, :])

    def transpose4(dst_sb):
        """ctx_full_sb [128b, 512d] -> dst_sb [128d(pc-tile), 4*128b]."""
        pt = psum_misc.tile([128, 512], F32, name="pt", tag="misc")
        for k in range(4):
            nc.tensor.transpose(pt[:, k * 128:(k + 1) * 128],
                                ctx_full_sb[:, k * 128:(k + 1) * 128],
                                eye32_sb[:, :])
        nc.vector.tensor_copy(dst_sb[:, :], pt[:, :])

    # ---- feats_mean: uniform-weight context, AG, transpose -> fmT_sb ----
    ctx_rounds(lambda b: uni_sb[0:128, 0:1], lambda b: uni_sb[0:68, 0:1])
    transpose4(fmT_sb)

    # ---- A'' precompute: 7 chunks of 512 over the 3584 free dim ----
    with tc.tile_pool(name="initp", bufs=2) as initp, \
         tc.tile_pool(name="initc", bufs=1) as initc:
        wvw_sb = initc.tile([128, 2048], BF, name="wvw_sb")
        wabs_sb = initc.tile([1, 512], BF, name="wabs_sb")
        nc.sync.dma_start(wvw_sb[:, :], io["wvw"][:, :])
        nc.sync.dma_start(wabs_sb[:, :], io["wabs"][:, :])
        for c7 in range(7):
            fchunk = initp.tile([128, 2048], BF, name="fchunk", tag="fchunk")
            fbias = initp.tile([1, 512], BF, name="fbias", tag="fbias")
            for k in range(4):
                nc.sync.dma_start(
                    fchunk[:, k * 512:(k + 1) * 512],
                    io["featT"][128 * k:128 * (k + 1), 512 * c7:512 * (c7 + 1)])
            nc.sync.dma_start(fbias[:, :],
                              io["featT"][512:513, 512 * c7:512 * (c7 + 1)])
            for m in range(4):
                p = psum_mm.tile([128, 512], F32, name="pA", tag="mm")
                for k in range(4):
                    nc.tensor.matmul(
                        p[:, :],
                        wvw_sb[:, k * 512 + 128 * m:k * 512 + 128 * m + 128],
                        fchunk[:, k * 512:(k + 1) * 512],
                        start=(k == 0), stop=False)
                nc.tensor.matmul(p[:, :],
                                 wabs_sb[0:1, 128 * m:128 * m + 128],
                                 fbias[0:1, :],
                                 start=False, stop=True)
                dst = A_sb[:, m * BNF + 512 * c7:m * BNF + 512 * (c7 + 1)]
                if m % 2 == 0:
                    nc.vector.tensor_copy(dst, p[:, :])
                else:
                    nc.scalar.copy(dst, p[:, :])

    # ---------------- per-step bodies ----------------
    def emit_wout(t):
        for (c0, c1) in ((0, 512), (512, 1024), (1024, VLP)):
            W = c1 - c0
            p = psum_mm.tile([128, 512], F32, name="pw", tag="mm")
            for k in range(8):
                nc.tensor.matmul(
                    p[:, 0:W],
                    hxT_bf[:, 128 * k:128 * (k + 1)],
                    wout_sb[:, k * VLP + c0:k * VLP + c1],
                    start=(k == 0), stop=False)
            nc.tensor.matmul(p[:, 0:W], ones_sb[0:1, :],
                             bout_sb[0:1, c0:c1], start=False, stop=True)
            nc.scalar.copy(out_sb[:, c0:c1], p[:, 0:W])
            nc.sync.dma_start(io["out"][t:t + 1, :, c0:c1], out_sb[:, c0:c1])

    import concourse.bass as bass_mod

    def emit_step(t):
        # ---- embedding prefetch (feature-major xT rows 512..1023) ----
        if t > 0:
            et = emb_pool.tile([128, 512], BF, name="et", tag="et")
            nc.sync.dma_start(et[:, :], io["embT"][t:t + 1, :, :])
        else:
            et = fmT_sb

        # ---- att_h'' (VD-major, own batches): [128, (m,b)] ----
        # rhs = own-batch columns of hxT_bf, selected by a per-core register
        hxTv = hxT_bf.rearrange("p (k b) -> p k b", k=8)
        pm = psum_misc.tile([128, 64], F32, name="pm", tag="misc")
        for m in range(4):
            for k in range(8):
                nc.tensor.matmul(
                    pm[:, m * 16:(m + 1) * 16],
                    hwfold_sb[:, k * 512 + 128 * m:k * 512 + 128 * m + 128],
                    hxTv[:, k, bass_mod.ts(pid_pe, BL)],
                    start=(k == 0), stop=(k == 7))
        nc.vector.tensor_copy(attT_sb[:, :], pm[:, :])

        # ---- vocab projection of previous step's hx (fills PE while DVE/ACT
        #      run the relu-add broadcast) ----
        if t > 0:
            emit_wout(t - 1)

        # ---- full'' = relu(A'' + att_h'' bcast over n) ----
        for m in range(4):
            for b in range(BL):
                col = m * 16 + b
                o = m * BNF + b * NP
                src = A_sb[:, o:o + 196]
                dst = full_sb[:, o:o + 196]
                bias = attT_sb[:, col:col + 1]
                if col % 4 == 3:
                    nc.scalar.activation(dst, src, AF.Relu, bias=bias, scale=1.0)
                else:
                    nc.vector.tensor_scalar(dst, src, bias, 0.0,
                                            op0=OP.add, op1=OP.max)

        # ---- scores (block-diagonal lhsT -> per-b psum rows) + one exp ----
        # wsgn_sb col block (m*16+b)*16..+16 is a [128,16] matrix whose only
        # nonzero column is b, holding sign(w) for d-tile m.
        ps = psum_sc.tile([16, NP], F32, name="ps", tag="sc")
        for b in range(BL):
            for m in range(4):
                nc.tensor.matmul(
                    ps[:, :], wsgn_sb[:, (m * 16 + b) * 16:(m * 16 + b + 1) * 16],
                    full_sb[:, m * BNF + b * NP:m * BNF + (b + 1) * NP],
                    start=(b == 0 and m == 0), stop=(b == 15 and m == 3))
        nc.scalar.activation(alpha_sb[:, 0:196], ps[:, 0:196], AF.Exp,
                             accum_out=sumexp_sb[:, 0:1])
        nc.vector.reciprocal(rs_sb[:, :], sumexp_sb[:, :])
        nc.vector.tensor_scalar_mul(alpha_sb[:, :], alpha_sb[:, :],
                                    rs_sb[:, 0:1])

        # ---- alpha^T (n-major) via PE transpose ----
        pa1 = psum_misc.tile([128, 16], BF, name="pa1", tag="misc")
        nc.tensor.transpose(pa1[:, :], alpha_sb[0:16, 0:128], eye16_sb[:, :])
        nc.vector.tensor_copy(aT1_sb[:, :], pa1[:, :])
        pa2 = psum_misc.tile([96, 16], BF, name="pa2", tag="misc")
        nc.tensor.transpose(pa2[:, :], alpha_sb[0:16, 128:224], eye16_sb[:, :])
        nc.vector.tensor_copy(aT2_sb[0:96, :], pa2[:, :])

        # ---- context (per-b matvec, col-group packed) + AllGather ----
        ctx_rounds(lambda b: aT1_sb[0:128, b:b + 1],
                   lambda b: aT2_sb[0:68, b:b + 1])

        # ---- gates = xT.T @ Wcat (K = [ctx 512 | emb 512 | hx 1024]) ----
        # emb/hx/bias K-tiles depend only on hxT(t-1)/embT, so they run on the
        # PE while the context AllGather is in flight; the ctx K-tiles follow
        # the post-AG transpose.
        pg = psum_mm.tile([128, 512], F32, name="pg", tag="mm")
        for k in range(4, 16):
            if k < 8:
                lhsT = et[:, (k - 4) * 128:(k - 3) * 128]
            else:
                lhsT = hxT_bf[:, (k - 8) * 128:(k - 7) * 128]
            nc.tensor.matmul(pg[:, :], lhsT,
                             wcat_sb[:, k * 512:(k + 1) * 512],
                             start=(k == 4), stop=False)
        nc.tensor.matmul(pg[:, :], ones_sb[0:1, :],
                         gbias_sb[0:1, :], start=False, stop=False)
        transpose4(ctxT_sb)
        for k in range(4):
            nc.tensor.matmul(pg[:, :], ctxT_sb[:, k * 128:(k + 1) * 128],
                             wcat_sb[:, k * 512:(k + 1) * 512],
                             start=False, stop=(k == 3))

        # ---- LSTM pointwise (gate cols: i 0:128 | f 128:256 | o 256:384 | g 384:512)
        #      sigmoid(x) = 0.5*tanh(x/2)+0.5 (keeps everything in exp table set)
        nc.scalar.activation(sig_sb[:, :], pg[:, 0:384], AF.Tanh, scale=0.5)
        nc.scalar.activation(gt_sb[:, :], pg[:, 384:512], AF.Tanh)
        nc.vector.tensor_scalar(sig_sb[:, :], sig_sb[:, :], 0.5, 0.5,
                                op0=OP.mult, op1=OP.add)
        nc.vector.tensor_tensor(tmp_sb[:, :], sig_sb[:, 0:128], gt_sb[:, :],
                                op=OP.mult)
        nc.vector.tensor_tensor(cx_sb[:, :], cx_sb[:, :], sig_sb[:, 128:256],
                                op=OP.mult)
        nc.vector.tensor_tensor(cx_sb[:, :], cx_sb[:, :], tmp_sb[:, :],
                                op=OP.add)
        nc.scalar.activation(tcx_sb[:, :], cx_sb[:, :], AF.Tanh)
        nc.vector.tensor_tensor(hx_sb[:, :], sig_sb[:, 256:384], tcx_sb[:, :],
                                op=OP.mult)

        # ---- hx^T piece + AllGather into hxT_sb ----
        ph = psum_misc.tile([128, 128], F32, name="ph", tag="misc")
        nc.tensor.transpose(ph[:, :], hx_sb[:, :], eye32_sb[:, :])
        nc.vector.tensor_copy(hxTp_sb[:, :], ph[:, :])
        nc.sync.dma_start(ag_hx_in[:, :], hxTp_sb[:, :])
        nc.gpsimd.collective_compute(
            "AllGather", OP.bypass, replica_groups=RG,
            ins=[ag_hx_in[:, :]], outs=[ag_hx_out[:, :]])
        for k in range(8):
            nc.sync.dma_start(hxT_sb[:, k * 128:(k + 1) * 128],
                              ag_hx_out[128 * k:128 * (k + 1), :])
        nc.vector.tensor_copy(hxT_bf[:, :], hxT_sb[:, :])

    for t in range(T):
        emit_step(t)
    emit_wout(T - 1)

    for cm in reversed(_pool_cms):
        cm.__exit__(None, None, None)
    _statics_cm.__exit__(None, None, None)


# ----------------------------------------------------------------------------
# host-side input preparation
# ----------------------------------------------------------------------------

def host_prep(inputs, T):
    """Returns (shared dict name->array, per-core list of dict name->array)."""
    f32 = np.float32
    features = np.asarray(inputs["features"], f32)           # [B,N,VD]
    captions = np.asarray(inputs["captions"]).astype(np.int64)  # [B,T]
    embed_table = np.asarray(inputs["embed_table"], f32)     # [V,E]
    W_ih = np.asarray(inputs["W_ih"], f32)                   # [4H, 2E]
    b_ih = np.asarray(inputs["b_ih"], f32)
    W_hh = np.asarray(inputs["W_hh"], f32)                   # [4H, H]
    b_hh = np.asarray(inputs["b_hh"], f32)
    W_out = np.asarray(inputs["W_out"], f32)                 # [V, H]
    b_out = np.asarray(inputs["b_out"], f32)
    att_vw_W = np.asarray(inputs["att_vw_W"], f32)           # [VD, VD]
    att_hw_W = np.asarray(inputs["att_hw_W"], f32)           # [VD, H]
    att_bias = np.asarray(inputs["att_bias"], f32)           # [N]
    att_w_W = np.asarray(inputs["att_w_W"], f32)             # [1, VD]

    w = att_w_W[0]
    wabs = np.abs(w)
    wsgn = np.sign(w).astype(f32)
    # sign(0) would drop the |w| fold; treat as +1 (relu(0*x)=0 anyway)
    wsgn[wsgn == 0] = 1.0

    shared = {}
    # wvw [128, 4*512]: [p, k*512+e] = wabs[e]*att_vw_W[e, 128k+p]
    wvw = (wabs[None, :] * att_vw_W.T)          # [d, e] = wabs[e]*W[e,d]
    shared["wvw"] = np.ascontiguousarray(
        wvw.reshape(4, 128, 512).transpose(1, 0, 2).reshape(128, 2048)
    ).astype(BF16)
    shared["wabs"] = wabs.reshape(1, 512).astype(BF16)
    # hwfold [128, 8*512]: [p, k*512+e] = wabs[e]*att_hw_W[e, 128k+p]
    hwf = (wabs[None, :] * att_hw_W.T)          # [h, e]
    shared["hwfold"] = np.ascontiguousarray(
        hwf.reshape(8, 128, 512).transpose(1, 0, 2).reshape(128, 4096)
    ).astype(BF16)
    # wsgn16 [128, (m,b,j)]: block-diagonal sign(w) selectors
    wsgnb = np.zeros((128, 4, 16, 16), np.float32)
    sgn_t = wsgn.reshape(4, 128).T                  # [p, m]
    for b in range(BL):
        wsgnb[:, :, b, b] = sgn_t
    shared["wsgn16"] = wsgnb.reshape(128, 1024).astype(BF16)
    # embT [T, 128, 512]: [t, p, k*128+b] = embed_table[captions[b, t-1], 128k+p]
    embT = np.zeros((T, 128, 512), BF16)
    if T > 1:
        emb = embed_table[captions[:, :T - 1]]       # [B, T-1, E]
        # -> [t-1, e, b] -> [t, p, k, b]
        embT[1:] = (emb.transpose(1, 2, 0)           # [T-1, E, B]
                    .reshape(T - 1, 4, 128, 128)
                    .transpose(0, 2, 1, 3)
                    .reshape(T - 1, 128, 512)).astype(BF16)
    shared["embT"] = embT
    shared["eye32"] = np.eye(128, dtype=f32)
    shared["eye16"] = np.eye(16, dtype=BF16)

    # LSTM gate bias (b_ih + b_hh) enters the gates matmul as a K=1 ones-row.
    gate_bias = b_ih + b_hh
    per_core = []
    for c in range(NCORES):
        d = {}
        bsl = slice(c * BL, (c + 1) * BL)
        featc = features[bsl]                    # [16, 196, 512]
        # featT [513, 3584]
        featT = np.zeros((513, 7 * 512), f32)
        ft = featc.transpose(2, 0, 1)            # [512, 16, 196]
        featT[:512] = np.pad(ft, ((0, 0), (0, 0), (0, NP - N))).reshape(512, BNF)
        featT[512] = np.tile(np.pad(att_bias, (0, NP - N)), BL)
        d["featT"] = featT.astype(BF16)
        # fea_nmaj [128, 16*2*512]
        fp = np.zeros((128, BL, 2, 512), f32)
        fc = np.pad(featc, ((0, 0), (0, 256 - N), (0, 0)))  # [16, 256, 512]
        fp[:, :, :, :] = fc.reshape(BL, 2, 128, 512).transpose(2, 0, 1, 3)
        d["fea"] = fp.reshape(128, 16384).astype(BF16)
        # wcat [128, 16*512]: gate col order [i, f, o, g] for h-slice c
        hsl = slice(c * HL, (c + 1) * HL)
        cols = np.concatenate([
            W_ih[0 * H:1 * H][hsl],      # i rows of W_ih -> [128, 2E]
            W_ih[1 * H:2 * H][hsl],      # f
            W_ih[3 * H:4 * H][hsl],      # o
            W_ih[2 * H:3 * H][hsl],      # g
        ], axis=0)                       # [512, 1024] (x part)
        colsh = np.concatenate([
            W_hh[0 * H:1 * H][hsl],
            W_hh[1 * H:2 * H][hsl],
            W_hh[3 * H:4 * H][hsl],
            W_hh[2 * H:3 * H][hsl],
        ], axis=0)                       # [512, 1024] (h part)
        wc = np.concatenate([cols.T, colsh.T], axis=0)   # [2048 K, 512 j]
        d["wcat"] = np.ascontiguousarray(
            wc.reshape(16, 128, 512).transpose(1, 0, 2).reshape(128, 8192)
        ).astype(BF16)
        d["gbias"] = np.concatenate([
            gate_bias[0 * H:1 * H][hsl], gate_bias[1 * H:2 * H][hsl],
            gate_bias[3 * H:4 * H][hsl], gate_bias[2 * H:3 * H][hsl],
        ]).reshape(1, 512).astype(BF16)
        # wout [128, 8*1280]
        vsl = slice(c * VL, (c + 1) * VL)
        wo = np.zeros((1024, VLP), f32)
        wo[:, :VL] = W_out[vsl].T
        d["wout"] = np.ascontiguousarray(
            wo.reshape(8, 128, VLP).transpose(1, 0, 2).reshape(128, 8 * VLP)
        ).astype(BF16)
        bo = np.zeros((1, VLP), f32)
        bo[0, :VL] = b_out[vsl]
        d["bout"] = bo.astype(BF16)
        d.update(shared)
        per_core.append(d)
    return per_core


IO_SPECS = [
    ("featT", [513, 3584], "bf16"),
    ("fea", [128, 16384], "bf16"),
    ("wvw", [128, 2048], "bf16"),
    ("wabs", [1, 512], "bf16"),
    ("hwfold", [128, 4096], "bf16"),
    ("wsgn16", [128, 1024], "bf16"),
    ("wcat", [128, 8192], "bf16"),
    ("wout", [128, 8 * VLP], "bf16"),
    ("bout", [1, VLP], "bf16"),
    ("eye32", [128, 128], "f32"),
    ("eye16", [16, 16], "bf16"),
    ("gbias", [1, 512], "bf16"),
]


def build_nc(T):
    from concourse import bass, bacc, tile, mybir
    nc = bacc.Bacc("TRN2", target_bir_lowering=False, debug=False,
                   num_devices=NCORES)
    io = {}
    for name, shape, dt in IO_SPECS:
        mdt = mybir.dt.float32 if dt == "f32" else mybir.dt.bfloat16
        io[name] = nc.dram_tensor(name, shape, mdt, kind="ExternalInput").ap()
    io["embT"] = nc.dram_tensor("embT", [T, 128, 512], mybir.dt.bfloat16,
                                kind="ExternalInput").ap()
    io["out"] = nc.dram_tensor("out", [T, 128, VLP], mybir.dt.float32,
                               kind="ExternalOutput").ap()
    with tile.TileContext(nc) as tc:
        build_program(tc, io, T)
    nc.compile()
    return nc


_CACHE = {}


def _run(inputs, T, trace=False):
    from concourse import bass_utils
    if T not in _CACHE:
        _CACHE[T] = build_nc(T)
    nc = _CACHE[T]
    in_maps = host_prep(inputs, T)
    res = bass_utils.run_bass_kernel_spmd(
        nc, in_maps, core_ids=list(range(NCORES)), trace=trace)
    return res


def assemble(res, T):
    out = np.empty((B, T, V), np.float32)
    for c in range(NCORES):
        o = res.results[c]["out"]                  # [T, 128, VLP]
        out[:, :, c * VL:(c + 1) * VL] = o[:, :, :VL].transpose(1, 0, 2)
    return out


def kernel(**inputs):
    T = int(np.asarray(inputs["captions"]).shape[1])
    res = _run(inputs, T)
    return assemble(res, T)


if __name__ == "__main__":
    pass



# revision 2
# speedup vs baseline: 22.0795x; 22.0795x over previous
"""Trainium2 Bass kernel for nn_DecoderRNN (soft-attention LSTM decoder).

Sharding (8 NeuronCores, one chip):
  - Attention (relu-add / scores / softmax / context): data-parallel over
    batch, 16 batches per core.  att-precomputed features stay SBUF-resident.
  - LSTM gates: tensor-parallel over hidden dim (128 h per core).
  - Vocab projection W_out: tensor-parallel over vocab (1250 v per core).
  - Per step: AllGather of context [B,VD] and of hx^T [H,B] across cores.

Layouts (per core):
  - A''  = |w| * (features @ att_vw_W.T + att_bias)  : [VD(4x128p), b*224+n] bf16
  - full'' = relu(A'' + |w|*att_h broadcast)          : same layout, bf16
  - scores = sum_e sign(w)[e] * full''[e, b, n]       : PE matvec (M=16 replicated)
  - context: per-b matvec over n, PE column-group packed 4-wide
  - gates/W_out: activations feature-major (xT) stationary, weights stream;
    bf16 matmul inputs, fp32 psum/pointwise state.
"""
import sys, os
sys.path.insert(0, "/opt/trn_rl_repo")

import numpy as np
import ml_dtypes

B, N, VD, H, E, V, T_FULL = 128, 196, 512, 1024, 512, 10000, 40
NCORES = 8
BL = B // NCORES          # 16 batches per core
NP = 224                  # padded n for free-dim layouts (16 chunks of 448 = 7x512-ish)
BNF = BL * NP             # 3584 free size of A''/full''
VL = 1250                 # true vocab per core
VLP = 1280                # padded vocab per core
HL = H // NCORES          # 128 hidden per core

BF16 = ml_dtypes.bfloat16


# ----------------------------------------------------------------------------
# device program
# ----------------------------------------------------------------------------

def build_program(tc, io, T):
    import concourse.bass as bass
    from concourse import mybir

    nc = tc.nc
    F32 = mybir.dt.float32
    BF = mybir.dt.bfloat16
    AF = mybir.ActivationFunctionType
    OP = mybir.AluOpType

    def f32r(ap):
        return ap.bitcast(mybir.dt.float32r)

    # ---------------- static SBUF tensors ----------------
    _statics_cm = tc.tile_pool(name="statics", bufs=1)
    _statics = _statics_cm.__enter__()

    def sb(name, shape, dt):
        return _statics.tile(shape, dt, name=name, tag=name)

    A_sb = sb("A_sb", [128, 4 * BNF], BF)
    full_sb = sb("full_sb", [128, 4 * BNF], BF)
    fea_sb = sb("fea_sb", [128, 16384], BF)
    wcat_sb = sb("wcat_sb", [128, 16 * 512], BF)
    wout_sb = sb("wout_sb", [128, 8 * VLP], BF)
    hwfold_sb = sb("hwfold_sb", [128, 8 * 512], BF)
    wsgn_sb = sb("wsgn_sb", [128, 1024], BF)
    hxT_bf = sb("hxT_bf", [128, 1024], BF)
    fmT_sb = sb("fmT_sb", [128, 512], BF)
    ctxT_sb = sb("ctxT_sb", [128, 512], BF)
    ctx_full_sb = sb("ctx_full_sb", [128, 512], BF)
    zcol_sb = sb("zcol_sb", [1, 128], BF)
    out_sb = sb("out_sb", [128, VLP], F32)
    attT_sb = sb("attT_sb", [128, 64], F32)
    hxTp_sb = sb("hxTp_sb", [128, 128], BF)
    eye128b_sb = sb("eye128b_sb", [128, 128], BF)
    alpha_sb = sb("alpha_sb", [16, NP], BF)
    sumexp_sb = sb("sumexp_sb", [16, 1], F32)
    rs_sb = sb("rs_sb", [16, 1], F32)
    aT1_sb = sb("aT1_sb", [128, 16], BF)
    aT2_sb = sb("aT2_sb", [128, 16], BF)
    sig_sb = sb("sig_sb", [128, 384], F32)
    gt_sb = sb("gt_sb", [128, 128], F32)
    tmp_sb = sb("tmp_sb", [128, 128], F32)
    tcx_sb = sb("tcx_sb", [128, 128], F32)
    hx_sb = sb("hx_sb", [128, 128], F32)
    cx_sb = sb("cx_sb", [128, 128], F32)
    eye32_sb = sb("eye32_sb", [128, 128], F32)
    eye16_sb = sb("eye16_sb", [16, 16], BF)
    ones_sb = sb("ones_sb", [1, 128], BF)
    uni_sb = sb("uni_sb", [128, 1], BF)
    bout_sb = sb("bout_sb", [1, VLP], BF)
    gbias_sb = sb("gbias_sb", [1, 512], BF)

    # ---------------- internal DRAM for collectives (bf16: consumers are
    # all bf16 matmul inputs, so the cast costs nothing) ----------------
    ag_ctx_in = nc.dram_tensor("ag_ctx_in", [BL, 512], BF, kind="Internal").ap()
    ag_ctx_out = nc.dram_tensor("ag_ctx_out", [128, 512], BF, kind="Internal",
                                addr_space="Shared").ap()
    ag_hx_in = nc.dram_tensor("ag_hx_in", [128, 128], BF, kind="Internal").ap()
    ag_hx_out = nc.dram_tensor("ag_hx_out", [1024, 128], BF, kind="Internal",
                               addr_space="Shared").ap()

    RG = [list(range(NCORES))]

    # ---------------- pools ----------------
    _pool_cms = []

    def open_pool(**kw):
        cm = tc.tile_pool(**kw)
        pool = cm.__enter__()
        _pool_cms.append(cm)
        return pool

    psum_mm = open_pool(name="psum_mm", bufs=2, space="PSUM")
    psum_ctx = open_pool(name="psum_ctx", bufs=2, space="PSUM")
    psum_sc = open_pool(name="psum_sc", bufs=2, space="PSUM")
    psum_misc = open_pool(name="psum_misc", bufs=2, space="PSUM")
    emb_pool = open_pool(name="emb_pool", bufs=2)
    scr_pool = open_pool(name="scr_pool", bufs=2)

    # ---------------- init: weight DMAs + memsets ----------------
    nc.sync.dma_start(fea_sb[:, :], io["fea"][:, :])
    nc.sync.dma_start(wcat_sb[:, :], io["wcat"][:, :])
    nc.sync.dma_start(wout_sb[:, :], io["wout"][:, :])
    nc.sync.dma_start(hwfold_sb[:, :], io["hwfold"][:, :])
    nc.sync.dma_start(wsgn_sb[:, :], io["wsgn16"][:, :])
    nc.sync.dma_start(eye32_sb[:, :], io["eye32"][:, :])
    nc.sync.dma_start(eye16_sb[:, :], io["eye16"][:, :])
    nc.sync.dma_start(eye128b_sb[:, :], io["eye128b"][:, :])
    nc.sync.dma_start(bout_sb[:, :], io["bout"][:, :])
    nc.sync.dma_start(gbias_sb[:, :], io["gbias"][:, :])

    nc.vector.memset(hxT_bf[:, :], 0.0)
    nc.vector.memset(cx_sb[:, :], 0.0)
    nc.vector.memset(full_sb[:, :], 0.0)
    nc.vector.memset(alpha_sb[:, :], 0.0)
    nc.vector.memset(ones_sb[:, :], 1.0)
    nc.vector.memset(uni_sb[:, :], 1.0 / N)
    nc.vector.memset(zcol_sb[:, :], 0.0)

    pid_pe = nc.tensor.partition_id()

    def ctx_rounds(lhs1, lhs2):
        """16 per-b matvecs over n (contract 196), 4 col-groups x 4 rounds.
        lhs1/lhs2: function b -> AP ([128,1] and [68,1] stationary vectors).
        Result rows land in psum rows 32j; cast to bf16 stage rows then
        DMA'd to ag_ctx_in rows 4r..4r+4."""
        for r in range(4):
            pc = psum_ctx.tile([128, 512], F32, name="pc", tag="pc")
            # K=1 zero matmul initializes every psum row (rows other than
            # 32j stay zero; the full-tile cast copy below reads them all)
            nc.tensor.matmul(pc[:, :], zcol_sb[0:1, :], fea_sb[0:1, 0:512],
                             start=True, stop=False, skip_group_check=True)
            for j in range(4):
                b = 4 * r + j
                nc.tensor.matmul(pc[32 * j:32 * j + 1, :], lhs1(b),
                                 fea_sb[0:128, (2 * b) * 512:(2 * b + 1) * 512],
                                 start=False, stop=False, tile_position=(0, 32 * j),
                                 skip_group_check=True)
                nc.tensor.matmul(pc[32 * j:32 * j + 1, :], lhs2(b),
                                 fea_sb[0:68, (2 * b + 1) * 512:(2 * b + 1) * 512 + 512],
                                 start=False, stop=(j == 3), tile_position=(0, 32 * j),
                                 skip_group_check=True)
            scr = scr_pool.tile([128, 512], BF, name="scr", tag="scr")
            if r % 2 == 0:
                nc.vector.tensor_copy(scr[:, :], pc[:, :])
            else:
                nc.scalar.copy(scr[:, :], pc[:, :])
            nc.sync.dma_start(ag_ctx_in[4 * r:4 * r + 4, :], scr[0:128:32, :])
        nc.gpsimd.collective_compute(
            "AllGather", OP.bypass, replica_groups=RG,
            ins=[ag_ctx_in[:, :]], outs=[ag_ctx_out[:, :]])
        nc.sync.dma_start(ctx_full_sb[:, :], ag_ctx_out[:, :])

    def transpose4(dst_sb):
        """ctx_full_sb [128b, 512d] -> dst_sb [128d(pc-tile), 4*128b]."""
        pt = psum_misc.tile([128, 512], BF, name="pt", tag="misc")
        for k in range(4):
            nc.tensor.transpose(pt[:, k * 128:(k + 1) * 128],
                                ctx_full_sb[:, k * 128:(k + 1) * 128],
                                eye128b_sb[:, :])
        nc.vector.tensor_copy(dst_sb[:, :], pt[:, :])

    # ---- feats_mean: uniform-weight context, AG, transpose -> fmT_sb ----
    ctx_rounds(lambda b: uni_sb[0:128, 0:1], lambda b: uni_sb[0:68, 0:1])
    transpose4(fmT_sb)

    # ---- A'' precompute: 7 chunks of 512 over the 3584 free dim ----
    with tc.tile_pool(name="initp", bufs=2) as initp, \
         tc.tile_pool(name="initc", bufs=1) as initc:
        wvw_sb = initc.tile([128, 2048], BF, name="wvw_sb")
        wabs_sb = initc.tile([1, 512], BF, name="wabs_sb")
        nc.sync.dma_start(wvw_sb[:, :], io["wvw"][:, :])
        nc.sync.dma_start(wabs_sb[:, :], io["wabs"][:, :])
        for c7 in range(7):
            fchunk = initp.tile([128, 2048], BF, name="fchunk", tag="fchunk")
            fbias = initp.tile([1, 512], BF, name="fbias", tag="fbias")
            for k in range(4):
                nc.sync.dma_start(
                    fchunk[:, k * 512:(k + 1) * 512],
                    io["featT"][128 * k:128 * (k + 1), 512 * c7:512 * (c7 + 1)])
            nc.sync.dma_start(fbias[:, :],
                              io["featT"][512:513, 512 * c7:512 * (c7 + 1)])
            for m in range(4):
                p = psum_mm.tile([128, 512], F32, name="pA", tag="mm")
                for k in range(4):
                    nc.tensor.matmul(
                        p[:, :],
                        wvw_sb[:, k * 512 + 128 * m:k * 512 + 128 * m + 128],
                        fchunk[:, k * 512:(k + 1) * 512],
                        start=(k == 0), stop=False)
                nc.tensor.matmul(p[:, :],
                                 wabs_sb[0:1, 128 * m:128 * m + 128],
                                 fbias[0:1, :],
                                 start=False, stop=True)
                dst = A_sb[:, m * BNF + 512 * c7:m * BNF + 512 * (c7 + 1)]
                if m % 2 == 0:
                    nc.vector.tensor_copy(dst, p[:, :])
                else:
                    nc.scalar.copy(dst, p[:, :])

    # ---------------- per-step bodies ----------------
    def emit_wout(t):
        for (c0, c1) in ((0, 512), (512, 1024), (1024, VLP)):
            W = c1 - c0
            p = psum_mm.tile([128, 512], F32, name="pw", tag="mm")
            for k in range(8):
                nc.tensor.matmul(
                    p[:, 0:W],
                    hxT_bf[:, 128 * k:128 * (k + 1)],
                    wout_sb[:, k * VLP + c0:k * VLP + c1],
                    start=(k == 0), stop=False)
            nc.tensor.matmul(p[:, 0:W], ones_sb[0:1, :],
                             bout_sb[0:1, c0:c1], start=False, stop=True)
            nc.scalar.copy(out_sb[:, c0:c1], p[:, 0:W])
            nc.sync.dma_start(io["out"][t:t + 1, :, c0:c1], out_sb[:, c0:c1])

    import concourse.bass as bass_mod

    def emit_step(t):
        # ---- embedding prefetch (feature-major xT rows 512..1023) ----
        if t > 0:
            et = emb_pool.tile([128, 512], BF, name="et", tag="et")
            nc.sync.dma_start(et[:, :], io["embT"][t:t + 1, :, :])
        else:
            et = fmT_sb

        # ---- att_h'' (VD-major, own batches): [128, (m,b)] ----
        # rhs = own-batch columns of hxT_bf, selected by a per-core register
        hxTv = hxT_bf.rearrange("p (k b) -> p k b", k=8)
        pm = psum_misc.tile([128, 64], F32, name="pm", tag="misc")
        for m in range(4):
            for k in range(8):
                nc.tensor.matmul(
                    pm[:, m * 16:(m + 1) * 16],
                    hwfold_sb[:, k * 512 + 128 * m:k * 512 + 128 * m + 128],
                    hxTv[:, k, bass_mod.ts(pid_pe, BL)],
                    start=(k == 0), stop=(k == 7))
        nc.vector.tensor_copy(attT_sb[:, :], pm[:, :])

        # ---- vocab projection of previous step's hx (fills PE while DVE/ACT
        #      run the relu-add broadcast) ----
        if t > 0:
            emit_wout(t - 1)

        # ---- full'' = relu(A'' + att_h'' bcast over n) ----
        # DVE/ACT split ~5:3 (DVE bf16 ~230ns/slice, ACT ~440ns/slice)
        for m in range(4):
            for b in range(BL):
                col = m * 16 + b
                o = m * BNF + b * NP
                src = A_sb[:, o:o + 196]
                dst = full_sb[:, o:o + 196]
                bias = attT_sb[:, col:col + 1]
                if col % 8 in (2, 5, 7):
                    nc.scalar.activation(dst, src, AF.Relu, bias=bias, scale=1.0)
                else:
                    nc.vector.tensor_scalar(dst, src, bias, 0.0,
                                            op0=OP.add, op1=OP.max)

        # ---- scores (block-diagonal lhsT -> per-b psum rows) + one exp ----
        # wsgn_sb col block (m*16+b)*16..+16 is a [128,16] matrix whose only
        # nonzero column is b, holding sign(w) for d-tile m.
        ps = psum_sc.tile([16, NP], F32, name="ps", tag="sc")
        for b in range(BL):
            for m in range(4):
                nc.tensor.matmul(
                    ps[:, :], wsgn_sb[:, (m * 16 + b) * 16:(m * 16 + b + 1) * 16],
                    full_sb[:, m * BNF + b * NP:m * BNF + (b + 1) * NP],
                    start=(b == 0 and m == 0), stop=(b == 15 and m == 3))
        nc.scalar.activation(alpha_sb[:, 0:196], ps[:, 0:196], AF.Exp,
                             accum_out=sumexp_sb[:, 0:1])
        nc.vector.reciprocal(rs_sb[:, :], sumexp_sb[:, :])
        nc.vector.tensor_scalar_mul(alpha_sb[:, :], alpha_sb[:, :],
                                    rs_sb[:, 0:1])

        # ---- alpha^T (n-major) via PE transpose ----
        pa1 = psum_misc.tile([128, 16], BF, name="pa1", tag="misc")
        nc.tensor.transpose(pa1[:, :], alpha_sb[0:16, 0:128], eye16_sb[:, :])
        nc.vector.tensor_copy(aT1_sb[:, :], pa1[:, :])
        pa2 = psum_misc.tile([96, 16], BF, name="pa2", tag="misc")
        nc.tensor.transpose(pa2[:, :], alpha_sb[0:16, 128:224], eye16_sb[:, :])
        nc.vector.tensor_copy(aT2_sb[0:96, :], pa2[:, :])

        # ---- context (per-b matvec, col-group packed) + AllGather ----
        ctx_rounds(lambda b: aT1_sb[0:128, b:b + 1],
                   lambda b: aT2_sb[0:68, b:b + 1])

        # ---- gates = xT.T @ Wcat (K = [ctx 512 | emb 512 | hx 1024]) ----
        # emb/hx/bias K-tiles depend only on hxT(t-1)/embT, so they run on the
        # PE while the context AllGather is in flight; the ctx K-tiles follow
        # the post-AG transpose.
        pg = psum_mm.tile([128, 512], F32, name="pg", tag="mm")
        for k in range(4, 16):
            if k < 8:
                lhsT = et[:, (k - 4) * 128:(k - 3) * 128]
            else:
                lhsT = hxT_bf[:, (k - 8) * 128:(k - 7) * 128]
            nc.tensor.matmul(pg[:, :], lhsT,
                             wcat_sb[:, k * 512:(k + 1) * 512],
                             start=(k == 4), stop=False)
        nc.tensor.matmul(pg[:, :], ones_sb[0:1, :],
                         gbias_sb[0:1, :], start=False, stop=False)
        transpose4(ctxT_sb)
        for k in range(4):
            nc.tensor.matmul(pg[:, :], ctxT_sb[:, k * 128:(k + 1) * 128],
                             wcat_sb[:, k * 512:(k + 1) * 512],
                             start=False, stop=(k == 3))

        # ---- LSTM pointwise (gate cols: i 0:128 | f 128:256 | o 256:384 | g 384:512)
        #      sigmoid(x) = 0.5*tanh(x/2)+0.5 (keeps everything in exp table set)
        nc.scalar.activation(sig_sb[:, :], pg[:, 0:384], AF.Tanh, scale=0.5)
        nc.scalar.activation(gt_sb[:, :], pg[:, 384:512], AF.Tanh)
        nc.vector.tensor_scalar(sig_sb[:, :], sig_sb[:, :], 0.5, 0.5,
                                op0=OP.mult, op1=OP.add)
        nc.vector.tensor_tensor(tmp_sb[:, :], sig_sb[:, 0:128], gt_sb[:, :],
                                op=OP.mult)
        nc.vector.tensor_tensor(cx_sb[:, :], cx_sb[:, :], sig_sb[:, 128:256],
                                op=OP.mult)
        nc.vector.tensor_tensor(cx_sb[:, :], cx_sb[:, :], tmp_sb[:, :],
                                op=OP.add)
        nc.scalar.activation(tcx_sb[:, :], cx_sb[:, :], AF.Tanh)
        nc.vector.tensor_tensor(hx_sb[:, :], sig_sb[:, 256:384], tcx_sb[:, :],
                                op=OP.mult)

        # ---- hx^T piece (bf16) + AllGather straight into hxT_bf ----
        ph = psum_misc.tile([128, 128], F32, name="ph", tag="misc")
        nc.tensor.transpose(ph[:, :], hx_sb[:, :], eye32_sb[:, :])
        nc.vector.tensor_copy(hxTp_sb[:, :], ph[:, :])
        nc.sync.dma_start(ag_hx_in[:, :], hxTp_sb[:, :])
        nc.gpsimd.collective_compute(
            "AllGather", OP.bypass, replica_groups=RG,
            ins=[ag_hx_in[:, :]], outs=[ag_hx_out[:, :]])
        nc.sync.dma_start(
            hxT_bf.rearrange("p (k j) -> p k j", k=8),
            ag_hx_out.rearrange("(k p) j -> p k j", p=128))

    for t in range(T):
        emit_step(t)
    emit_wout(T - 1)

    for cm in reversed(_pool_cms):
        cm.__exit__(None, None, None)
    _statics_cm.__exit__(None, None, None)


# ----------------------------------------------------------------------------
# host-side input preparation
# ----------------------------------------------------------------------------

def host_prep(inputs, T):
    """Returns (shared dict name->array, per-core list of dict name->array)."""
    f32 = np.float32
    features = np.asarray(inputs["features"], f32)           # [B,N,VD]
    captions = np.asarray(inputs["captions"]).astype(np.int64)  # [B,T]
    embed_table = np.asarray(inputs["embed_table"], f32)     # [V,E]
    W_ih = np.asarray(inputs["W_ih"], f32)                   # [4H, 2E]
    b_ih = np.asarray(inputs["b_ih"], f32)
    W_hh = np.asarray(inputs["W_hh"], f32)                   # [4H, H]
    b_hh = np.asarray(inputs["b_hh"], f32)
    W_out = np.asarray(inputs["W_out"], f32)                 # [V, H]
    b_out = np.asarray(inputs["b_out"], f32)
    att_vw_W = np.asarray(inputs["att_vw_W"], f32)           # [VD, VD]
    att_hw_W = np.asarray(inputs["att_hw_W"], f32)           # [VD, H]
    att_bias = np.asarray(inputs["att_bias"], f32)           # [N]
    att_w_W = np.asarray(inputs["att_w_W"], f32)             # [1, VD]

    w = att_w_W[0]
    wabs = np.abs(w)
    wsgn = np.sign(w).astype(f32)
    # sign(0) would drop the |w| fold; treat as +1 (relu(0*x)=0 anyway)
    wsgn[wsgn == 0] = 1.0

    shared = {}
    # wvw [128, 4*512]: [p, k*512+e] = wabs[e]*att_vw_W[e, 128k+p]
    wvw = (wabs[None, :] * att_vw_W.T)          # [d, e] = wabs[e]*W[e,d]
    shared["wvw"] = np.ascontiguousarray(
        wvw.reshape(4, 128, 512).transpose(1, 0, 2).reshape(128, 2048)
    ).astype(BF16)
    shared["wabs"] = wabs.reshape(1, 512).astype(BF16)
    # hwfold [128, 8*512]: [p, k*512+e] = wabs[e]*att_hw_W[e, 128k+p]
    hwf = (wabs[None, :] * att_hw_W.T)          # [h, e]
    shared["hwfold"] = np.ascontiguousarray(
        hwf.reshape(8, 128, 512).transpose(1, 0, 2).reshape(128, 4096)
    ).astype(BF16)
    # wsgn16 [128, (m,b,j)]: block-diagonal sign(w) selectors
    wsgnb = np.zeros((128, 4, 16, 16), np.float32)
    sgn_t = wsgn.reshape(4, 128).T                  # [p, m]
    for b in range(BL):
        wsgnb[:, :, b, b] = sgn_t
    shared["wsgn16"] = wsgnb.reshape(128, 1024).astype(BF16)
    # embT [T, 128, 512]: [t, p, k*128+b] = embed_table[captions[b, t-1], 128k+p]
    embT = np.zeros((T, 128, 512), BF16)
    if T > 1:
        emb = embed_table[captions[:, :T - 1]]       # [B, T-1, E]
        # -> [t-1, e, b] -> [t, p, k, b]
        embT[1:] = (emb.transpose(1, 2, 0)           # [T-1, E, B]
                    .reshape(T - 1, 4, 128, 128)
                    .transpose(0, 2, 1, 3)
                    .reshape(T - 1, 128, 512)).astype(BF16)
    shared["embT"] = embT
    shared["eye32"] = np.eye(128, dtype=f32)
    shared["eye16"] = np.eye(16, dtype=BF16)
    shared["eye128b"] = np.eye(128, dtype=BF16)

    # LSTM gate bias (b_ih + b_hh) enters the gates matmul as a K=1 ones-row.
    gate_bias = b_ih + b_hh
    per_core = []
    for c in range(NCORES):
        d = {}
        bsl = slice(c * BL, (c + 1) * BL)
        featc = features[bsl]                    # [16, 196, 512]
        # featT [513, 3584]
        featT = np.zeros((513, 7 * 512), f32)
        ft = featc.transpose(2, 0, 1)            # [512, 16, 196]
        featT[:512] = np.pad(ft, ((0, 0), (0, 0), (0, NP - N))).reshape(512, BNF)
        featT[512] = np.tile(np.pad(att_bias, (0, NP - N)), BL)
        d["featT"] = featT.astype(BF16)
        # fea_nmaj [128, 16*2*512]
        fp = np.zeros((128, BL, 2, 512), f32)
        fc = np.pad(featc, ((0, 0), (0, 256 - N), (0, 0)))  # [16, 256, 512]
        fp[:, :, :, :] = fc.reshape(BL, 2, 128, 512).transpose(2, 0, 1, 3)
        d["fea"] = fp.reshape(128, 16384).astype(BF16)
        # wcat [128, 16*512]: gate col order [i, f, o, g] for h-slice c
        hsl = slice(c * HL, (c + 1) * HL)
        cols = np.concatenate([
            W_ih[0 * H:1 * H][hsl],      # i rows of W_ih -> [128, 2E]
            W_ih[1 * H:2 * H][hsl],      # f
            W_ih[3 * H:4 * H][hsl],      # o
            W_ih[2 * H:3 * H][hsl],      # g
        ], axis=0)                       # [512, 1024] (x part)
        colsh = np.concatenate([
            W_hh[0 * H:1 * H][hsl],
            W_hh[1 * H:2 * H][hsl],
            W_hh[3 * H:4 * H][hsl],
            W_hh[2 * H:3 * H][hsl],
        ], axis=0)                       # [512, 1024] (h part)
        wc = np.concatenate([cols.T, colsh.T], axis=0)   # [2048 K, 512 j]
        d["wcat"] = np.ascontiguousarray(
            wc.reshape(16, 128, 512).transpose(1, 0, 2).reshape(128, 8192)
        ).astype(BF16)
        d["gbias"] = np.concatenate([
            gate_bias[0 * H:1 * H][hsl], gate_bias[1 * H:2 * H][hsl],
            gate_bias[3 * H:4 * H][hsl], gate_bias[2 * H:3 * H][hsl],
        ]).reshape(1, 512).astype(BF16)
        # wout [128, 8*1280]
        vsl = slice(c * VL, (c + 1) * VL)
        wo = np.zeros((1024, VLP), f32)
        wo[:, :VL] = W_out[vsl].T
        d["wout"] = np.ascontiguousarray(
            wo.reshape(8, 128, VLP).transpose(1, 0, 2).reshape(128, 8 * VLP)
        ).astype(BF16)
        bo = np.zeros((1, VLP), f32)
        bo[0, :VL] = b_out[vsl]
        d["bout"] = bo.astype(BF16)
        d.update(shared)
        per_core.append(d)
    return per_core


IO_SPECS = [
    ("featT", [513, 3584], "bf16"),
    ("fea", [128, 16384], "bf16"),
    ("wvw", [128, 2048], "bf16"),
    ("wabs", [1, 512], "bf16"),
    ("hwfold", [128, 4096], "bf16"),
    ("wsgn16", [128, 1024], "bf16"),
    ("wcat", [128, 8192], "bf16"),
    ("wout", [128, 8 * VLP], "bf16"),
    ("bout", [1, VLP], "bf16"),
    ("eye32", [128, 128], "f32"),
    ("eye16", [16, 16], "bf16"),
    ("eye128b", [128, 128], "bf16"),
    ("gbias", [1, 512], "bf16"),
]


def build_nc(T):
    from concourse import bass, bacc, tile, mybir
    nc = bacc.Bacc("TRN2", target_bir_lowering=False, debug=False,
                   num_devices=NCORES)
    io = {}
    for name, shape, dt in IO_SPECS:
        mdt = mybir.dt.float32 if dt == "f32" else mybir.dt.bfloat16
        io[name] = nc.dram_tensor(name, shape, mdt, kind="ExternalInput").ap()
    io["embT"] = nc.dram_tensor("embT", [T, 128, 512], mybir.dt.bfloat16,
                                kind="ExternalInput").ap()
    io["out"] = nc.dram_tensor("out", [T, 128, VLP], mybir.dt.float32,
                               kind="ExternalOutput").ap()
    with tile.TileContext(nc) as tc:
        build_program(tc, io, T)
    nc.compile()
    return nc


_CACHE = {}


def _run(inputs, T, trace=False):
    from concourse import bass_utils
    if T not in _CACHE:
        _CACHE[T] = build_nc(T)
    nc = _CACHE[T]
    in_maps = host_prep(inputs, T)
    res = bass_utils.run_bass_kernel_spmd(
        nc, in_maps, core_ids=list(range(NCORES)), trace=trace)
    return res


def assemble(res, T):
    out = np.empty((B, T, V), np.float32)
    for c in range(NCORES):
        o = res.results[c]["out"]                  # [T, 128, VLP]
        out[:, :, c * VL:(c + 1) * VL] = o[:, :, :VL].transpose(1, 0, 2)
    return out


def kernel(**inputs):
    T = int(np.asarray(inputs["captions"]).shape[1])
    res = _run(inputs, T)
    return assemble(res, T)


if __name__ == "__main__":
    pass

